# revision 83
# baseline (speedup 1.0000x reference)
"""Trainium2 Bass kernel for nn_Attention_18726057410905.

Multi-head causal attention: B=8, S=1024, D=768, N=12 heads, H=64.
Sharding: data-parallel over batch -- core b computes batch element b.
No collectives.

Per-core dataflow (all matmul inputs bf16, fp32 PSUM accumulation):
  x^T   [d,s]   via PE identity-transposes of fp32 x; bf16 cast on evacuation
  Q^T,K^T [2*64h, s] per head-pair (W stationary, x^T moving)
  V_aug [s, n, 128]  natural layout + 64-wide ones block (cols 64:128)
  S^T   [k-tile 128, 2 halves x 512q] -- one 2-bank PSUM tile per k-tile,
        2 heads row-packed on the PE (K=64 contraction)
  P^T   = exp(S^T/8) via one ACT activation per k-tile; triangular mask
          (gpsimd) on diagonal tiles only; fully-masked tiles never computed
  z_aug^T [128, q] = sum_k V_aug.T @ P^T; rows 64:128 hold the softmax
        denominators replicated by the ones block (broadcast for free)
  z^T normalized with an all-DVE copy/reciprocal_approx_fast/multiply chain
  out   [q, e] = z^T.T @ W_O + b_O

Pipelining: PV matmuls trail S^T/exp by LOOKAHEAD k-tiles; the next pair's
Q/K projection matmuls (and, for the last pair, the output projection) are
drip-fed into the attention stream as PE filler so the in-order PE never
idles on the ACT exp stream and the HAM clock-gate stays at full rate.
"""

from contextlib import ExitStack

import numpy as np

import concourse.bass as bass
import concourse.tile as tile
from concourse import bacc, mybir
from concourse.bass_utils import run_bass_kernel_spmd
from concourse.masks import make_identity, make_upper_triangular
from concourse.tile_rust import add_dep_helper

B, S, D, N, H = 8, 1024, 768, 12, 64
P = 128
N_CORES = 8
DT = D // P          # 6 d-tiles
NPAIR = N // 2       # 6 head pairs
QB = 512             # q-block width
SB = S // QB         # 2 q/s blocks
KT = S // P          # 8 k/s tiles
EB = 384             # e-block width for the output projection
LOOKAHEAD = 4        # k-tiles of PV deferral (keeps PE fed while ACT exps)
BF16 = mybir.dt.bfloat16
F32 = mybir.dt.float32
AF = mybir.ActivationFunctionType
ALU = mybir.AluOpType


def _build_nc():
    nc = bacc.Bacc(
        "TRN2", target_bir_lowering=False, debug=False, num_devices=N_CORES
    )
    x_d = nc.dram_tensor("x", [S, D], F32, kind="ExternalInput").ap()
    wq_d = nc.dram_tensor("wq", [N, D, H], F32, kind="ExternalInput").ap()
    wk_d = nc.dram_tensor("wk", [N, D, H], F32, kind="ExternalInput").ap()
    wv_d = nc.dram_tensor("wv", [N, D, H], F32, kind="ExternalInput").ap()
    wo_d = nc.dram_tensor("wo", [N, H, D], F32, kind="ExternalInput").ap()
    bq_d = nc.dram_tensor("bq", [N, H], F32, kind="ExternalInput").ap()
    bk_d = nc.dram_tensor("bk", [N, H], F32, kind="ExternalInput").ap()
    bv_d = nc.dram_tensor("bv", [N, H], F32, kind="ExternalInput").ap()
    bo_d = nc.dram_tensor("bo", [D], F32, kind="ExternalInput").ap()
    out_d = nc.dram_tensor("out", [S, D], F32, kind="ExternalOutput").ap()

    with tile.TileContext(nc) as tc, ExitStack() as ctx:
        _body(ctx, tc, x_d, wq_d, wk_d, wv_d, wo_d, bq_d, bk_d, bv_d, bo_d, out_d)
    nc.compile()
    return nc


def _body(ctx, tc, x_d, wq_d, wk_d, wv_d, wo_d, bq_d, bk_d, bv_d, bo_d, out_d):
    nc = tc.nc
    const = ctx.enter_context(tc.tile_pool(name="const", bufs=1))
    wstage = ctx.enter_context(tc.tile_pool(name="wstage", bufs=2))
    xstage = ctx.enter_context(tc.tile_pool(name="xstage", bufs=3))
    ppool = ctx.enter_context(tc.tile_pool(name="ppool", bufs=8))
    spool = ctx.enter_context(tc.tile_pool(name="spool", bufs=4))
    opool = ctx.enter_context(tc.tile_pool(name="opool", bufs=4))
    dram = ctx.enter_context(tc.tile_pool(name="dram", bufs=1, space="DRAM"))
    ps_mm = ctx.enter_context(tc.tile_pool(name="ps_mm", bufs=3, space="PSUM"))
    ps_pj = ps_mm
    ps_z = ctx.enter_context(tc.tile_pool(name="ps_z", bufs=2, space="PSUM"))

    # --- engine warmups ----------------------------------------------------
    # DVE pays ~11us on its first real op; ACT pays a ~2.7us exp-table load.
    # Absorb both at t=0, concurrent with the input DMAs.
    warm = const.tile([1, 8], F32, tag="warm")
    nc.vector.memset(warm[:], 1.0)
    warmp = ps_z.tile([1, 8], F32, tag="z", name="warmp")
    nc.vector.tensor_copy(warmp[:], warm[:])
    warmb = const.tile([1, 8], BF16, tag="warmb")
    nc.vector.tensor_copy(warmb[:], warmp[:])  # preload DVE psum-read CAST path
    nc.scalar.activation(warm[:], warm[:], AF.Exp, scale=1.0)

    # --- constants ---------------------------------------------------------
    # trimask[r, c] = 1 if r <= c else 0 (keep k <= q in [k, q] layout)
    trimask = const.tile([P, P], BF16, tag="trimask")
    make_upper_triangular(nc, trimask[:], val=1.0, diag=True)

    bq_sb = const.tile([P, NPAIR], F32, tag="bq")
    nc.gpsimd.dma_start(bq_sb[:], bq_d.rearrange("(pr two) h -> (two h) pr", two=2))
    bk_sb = const.tile([P, NPAIR], F32, tag="bk")
    nc.gpsimd.dma_start(bk_sb[:], bk_d.rearrange("(pr two) h -> (two h) pr", two=2))
    bv_rep = const.tile([P, N * H], F32, tag="bvrep")
    nc.gpsimd.dma_start(
        bv_rep[:], bv_d.rearrange("n h -> (n h)")[None, :].to_broadcast((P, N * H))
    )
    bo_rep = const.tile([P, D], F32, tag="borep")
    nc.gpsimd.dma_start(bo_rep[:], bo_d[None, :].to_broadcast((P, D)))

    # wq/wk layout: [N, D, H] -> [128 dp, NPAIR, DT, (n2 h)]
    wq_sb = const.tile([P, NPAIR, DT, P], BF16, tag="wq")
    wk_sb = const.tile([P, NPAIR, DT, P], BF16, tag="wk")
    wq_r = wq_d.rearrange("n (dt dp) h -> n dp dt h", dp=P)
    wk_r = wk_d.rearrange("n (dt dp) h -> n dp dt h", dp=P)

    def stage_qk_pair(pr, eng=None):
        eng = eng or nc.sync
        out = []
        for w_r, nm in ((wq_r, "q"), (wk_r, "k")):
            stg = wstage.tile([P, DT, P], F32, tag="wpstg", name=f"stg{nm}{pr}")
            for a in range(2):
                eng.dma_start(stg[:, :, bass.ts(a, H)], w_r[2 * pr + a])
            out.append(stg)
        return out

    def cast_qk_pair(pr, stgs):
        nc.vector.tensor_copy(wq_sb[:, pr], stgs[0][:])
        nc.vector.tensor_copy(wk_sb[:, pr], stgs[1][:])

    # --- x load + PE transpose -> xT [128 dp, DT, S] ----------------------
    # PE transposes fp32 x tiles directly (identity matmul); the PSUM->SBUF
    # evacuation casts to bf16. No XBAR DMAs, and the PE warms up early.
    ident = const.tile([P, P], F32, tag="ident")
    make_identity(nc, ident[:])
    xT = const.tile([P, DT, S], BF16, tag="xT")
    stg0 = stage_qk_pair(0, nc.scalar)  # scalar queue: lands with the x loads
    xs_last = None
    for st in range(KT):
        xs = xstage.tile([P, D], F32, tag="xs")
        nc.sync.dma_start(xs[:], x_d[bass.ts(st, P), :])
        xs_last = xs
        for dt in range(DT):
            pt = ps_mm.tile([P, 2 * QB], F32, tag="mm", name=f"xtr_{st}_{dt}")
            nc.tensor.transpose(pt[:, 0:P], xs[:, bass.ts(dt, P)], ident[:])
            nc.vector.tensor_copy(xT[:, dt, bass.ts(st, P)], pt[:, 0:P])
    cast_qk_pair(0, stg0)
    # 4-byte bounce read of the last x tile: the bulk weight DMAs queued
    # behind it on the in-order sync engine start only after x (and the
    # pair-0 weights on the scalar queue) have had exclusive DMA bandwidth
    xgate = const.tile([1, 1], F32, tag="xgate")
    nc.sync.dma_start(xgate[:], xs_last[0:1, 0:1])

    # --- weight loads (f32 stage -> bf16) ---------------------------------
    def load_w_dh(w_d, tag, eng):
        # [N, D, H] -> [128 dp, DT, (pr n2 h)]; one 4D DMA + one cast per dt
        # so the first consumer matmuls don't wait for the whole weight.
        stg = wstage.tile([P, DT, N * H], F32, tag="wstg", name=f"stg_{tag}")
        wsb = const.tile([P, DT, N * H], BF16, tag=tag)
        w_r = w_d.rearrange("(pr a) (dt dp) h -> dt dp pr a h", a=2, dp=P)
        for dt in range(DT):
            eng.dma_start(
                stg[:, dt, :].rearrange("p (pr a b) -> p pr a b", pr=NPAIR, a=2),
                w_r[dt],
            )
            nc.vector.tensor_copy(wsb[:, dt, :], stg[:, dt, :])
        return wsb

    wv_sb = load_w_dh(wv_d, "wv", nc.sync)
    for pr in range(1, NPAIR):
        cast_qk_pair(pr, stage_qk_pair(pr))

    qT = const.tile([P, NPAIR, S], BF16, tag="qT")
    kT = const.tile([P, NPAIR, S], BF16, tag="kT")
    zT = const.tile([P, NPAIR, S], BF16, tag="zT")
    # V_aug: the 64-wide ones block makes the PV matmul replicate the softmax
    # denominators into PSUM partitions 64:128 -- broadcast for free.
    v_aug = const.tile([P, KT, N, 2 * H], BF16, tag="vaug")
    nc.gpsimd.memset(v_aug[:], 1.0)

    def qk_proj_steps(pr):
        # Q^T (bank 0) and K^T (bank 1) of one 2-bank psum tile, as a list of
        # single-matmul closures so the pair-(pr) projection can be drip-fed
        # into pair-(pr-1)'s attention stream as PE filler work.
        steps = []
        for sb_i in range(SB):
            box = {}

            def mk(dt, half, sb_i=sb_i, box=box):
                def go():
                    if "t" not in box:
                        box["t"] = ps_pj.tile(
                            [P, 2 * QB], F32, tag="mm", name=f"pqk_{pr}_{sb_i}"
                        )
                    pqk = box["t"]
                    wsb = wq_sb if half == 0 else wk_sb
                    nc.tensor.matmul(
                        pqk[:, half * QB : (half + 1) * QB],
                        lhsT=wsb[:, pr, dt, :],
                        rhs=xT[:, dt, bass.ts(sb_i, QB)],
                        start=(dt == 0),
                        stop=(dt == DT - 1),
                    )
                    if half == 1 and dt == DT - 1:
                        nc.vector.tensor_scalar_add(
                            qT[:, pr, bass.ts(sb_i, QB)], pqk[:, 0:QB],
                            bq_sb[:, pr : pr + 1],
                        )
                        nc.vector.tensor_scalar_add(
                            kT[:, pr, bass.ts(sb_i, QB)], pqk[:, QB : 2 * QB],
                            bk_sb[:, pr : pr + 1],
                        )

                return go

            for half in range(2):
                for dt in range(DT):
                    steps.append(mk(dt, half))
        return steps

    def qk_proj(pr):
        for s in qk_proj_steps(pr):
            s()

    def v_proj(sts):
        for st in sts:
            for blk in range(2):  # nh blocks of 384 = 6 heads
                pv = ps_mm.tile([P, 2 * QB], F32, tag="mm", name=f"pv_{st}_{blk}")
                for dt in range(DT):
                    nc.tensor.matmul(
                        pv[:, :EB],
                        lhsT=xT[:, dt, bass.ts(st, P)],
                        rhs=wv_sb[:, dt, bass.ts(blk, EB)],
                        start=(dt == 0),
                        stop=(dt == DT - 1),
                    )
                nc.vector.tensor_tensor(
                    v_aug[:, st, bass.ts(blk, 6), 0:H],
                    pv[:, :EB].rearrange("p (n h) -> p n h", h=H),
                    bv_rep[:, bass.ts(blk, EB)].rearrange("p (n h) -> p n h", h=H),
                    ALU.add,
                )

    # --- Q/K projections + attention, per head-pair -----------------------
    # --- W_O load: [N, H, D] -> [128 (n2 h), NPAIR, D] --------------------
    wo_stg = wstage.tile([P, NPAIR, D], F32, tag="wstg", name="stg_wo")
    for pr in range(NPAIR):
        nc.sync.dma_start(
            wo_stg[:, pr, :],
            wo_d[2 * pr : 2 * pr + 2].rearrange("n h e -> (n h) e"),
        )
    wo_sb = const.tile([P, NPAIR, D], BF16, tag="wo")
    nc.vector.tensor_copy(wo_sb[:], wo_stg[:])

    def o_proj_steps(qts, alt_pool=False):
        # out[q, e] = z^T.T @ W_O + b_O, as single-matmul closures
        steps = []
        for qt in qts:
            for eb in range(D // EB):
                box = {}

                def mk(pr, qt=qt, eb=eb, box=box):
                    def go():
                        if "t" not in box:
                            # tail-only: alternate into the attention z-pool
                            # (free after the last pair) for deeper rotation
                            if alt_pool and (2 * qt + eb) % 2:
                                box["t"] = ps_z.tile(
                                    [P, QB], F32, tag="z", name=f"po_{qt}_{eb}"
                                )
                            else:
                                box["t"] = ps_mm.tile(
                                    [P, 2 * QB], F32, tag="mm", name=f"po_{qt}_{eb}"
                                )
                        po = box["t"]
                        nc.tensor.matmul(
                            po[:, :EB],
                            lhsT=zT[:, pr, bass.ts(qt, P)],
                            rhs=wo_sb[:, pr, bass.ts(eb, EB)],
                            start=(pr == 0),
                            stop=(pr == NPAIR - 1),
                        )
                        if pr == NPAIR - 1:
                            ot = opool.tile([P, EB], F32, tag="ot")
                            nc.vector.tensor_tensor(
                                ot[:], po[:, :EB], bo_rep[:, bass.ts(eb, EB)],
                                ALU.add,
                            )
                            nc.sync.dma_start(
                                out_d[bass.ts(qt, P), bass.ts(eb, EB)], ot[:]
                            )

                    return go

                for pr in range(NPAIR):
                    steps.append(mk(pr))
        return steps

    o_first = o_proj_steps(range(4))  # q-tiles 0-3: fills attn(last, j=1)
    o_idx = [0]

    qk_proj(0)
    v_proj(range(KT))

    for pr in range(NPAIR):
        # next pair's projection matmuls drip-fed into this pair's attention;
        # the last pair's j=1 stream instead pulls output-projection matmuls
        last = pr + 1 >= NPAIR
        fill = qk_proj_steps(pr + 1) if not last else None
        fill_i = [0]

        def emit_fill(k=1):
            for _ in range(k):
                if fill is not None:
                    if fill_i[0] < len(fill):
                        fill[fill_i[0]]()
                        fill_i[0] += 1
                elif cur_j[0] == 1 and o_idx[0] < len(o_first):
                    o_first[o_idx[0]]()
                    o_idx[0] += 1

        cur_j = [0]
        # attention for the pair (2 heads row-packed on the PE)
        for j in range(SB):
            cur_j[0] = j
            n_kt = 4 * (j + 1)
            pz = [
                ps_z.tile([P, QB], F32, tag="z", name=f"z_{pr}_{j}_{h}")
                for h in range(2)
            ]
            pts = {}

            def emit_st(i):
                # S^T for both halves into one 2-bank tile; exp; mask
                q_off = max(0, (i - 4 * j) * P)
                ps = ps_mm.tile([P, 2 * QB], F32, tag="mm", name=f"s_{pr}_{j}_{i}")
                for half in range(2):
                    lo, hi = 64 * half, 64 * half + 64
                    nc.tensor.matmul(
                        ps[:, half * QB + q_off : (half + 1) * QB],
                        lhsT=kT[lo:hi, pr, bass.ts(i, P)],
                        rhs=qT[lo:hi, pr, j * QB + q_off : (j + 1) * QB],
                        start=True,
                        stop=True,
                    )
                pT = ppool.tile([P, 2, QB], BF16, tag="pT")
                ps3 = ps.rearrange("p (h q) -> p h q", h=2)
                nc.scalar.activation(
                    pT[:, :, q_off:], ps3[:, :, q_off:], AF.Exp, scale=0.125
                )
                if i >= 4 * j:  # diagonal tile: triangular mask, both halves
                    nc.gpsimd.tensor_tensor(
                        pT[:, :, q_off : q_off + P],
                        pT[:, :, q_off : q_off + P],
                        trimask[:, None, :].to_broadcast((P, 2, P)),
                        ALU.mult,
                    )
                pts[i] = pT

            def emit_pv(i):
                q_off = max(0, (i - 4 * j) * P)
                for half in range(2):
                    n = 2 * pr + half
                    nc.tensor.matmul(
                        pz[half][:, q_off:],
                        lhsT=v_aug[:, i, n, :],
                        rhs=pts[i][:, half, q_off:],
                        start=(i == 0),
                        stop=(i == n_kt - 1),
                    )

            for i in range(n_kt):
                emit_st(i)
                emit_fill()
                if i >= LOOKAHEAD:
                    emit_pv(i - LOOKAHEAD)
                    emit_fill()
            for i in range(max(0, n_kt - LOOKAHEAD), n_kt):
                emit_pv(i)
                emit_fill()

            # normalize z and store z^T (all-DVE chain; PSUM rows 64:128
            # hold the denominators replicated by the ones block)
            for half in range(2):
                lo, hi = 64 * half, 64 * half + 64
                sm = spool.tile([64, QB], F32, tag="sm")
                nc.vector.tensor_copy(sm[:], pz[half][H : 2 * H, :])
                rc = spool.tile([64, QB], F32, tag="rc")
                nc.vector.reciprocal_approx_fast(rc[:], sm[:])
                nc.vector.tensor_mul(
                    zT[lo:hi, pr, bass.ts(j, QB)], pz[half][0:H, :], rc[:]
                )

    # --- output projection leftovers -----------------------------------
    for s in o_first[o_idx[0] :]:
        s()
    for s in o_proj_steps(range(4, KT), alt_pool=True):
        s()


_CACHE = {}


def get_nc():
    if "nc" not in _CACHE:
        _CACHE["nc"] = _build_nc()
    return _CACHE["nc"]


def kernel(normalized_resid_pre, W_Q, W_K, W_V, W_O, b_Q, b_K, b_V, b_O, **kw):
    x = np.ascontiguousarray(np.asarray(normalized_resid_pre, dtype=np.float32))
    shared = {
        "wq": np.ascontiguousarray(np.asarray(W_Q, dtype=np.float32)),
        "wk": np.ascontiguousarray(np.asarray(W_K, dtype=np.float32)),
        "wv": np.ascontiguousarray(np.asarray(W_V, dtype=np.float32)),
        "wo": np.ascontiguousarray(np.asarray(W_O, dtype=np.float32)),
        "bq": np.ascontiguousarray(np.asarray(b_Q, dtype=np.float32)),
        "bk": np.ascontiguousarray(np.asarray(b_K, dtype=np.float32)),
        "bv": np.ascontiguousarray(np.asarray(b_V, dtype=np.float32)),
        "bo": np.ascontiguousarray(np.asarray(b_O, dtype=np.float32)),
    }
    in_maps = [dict(shared, x=x[b]) for b in range(B)]
    nc = get_nc()
    res = run_bass_kernel_spmd(nc, in_maps, core_ids=list(range(N_CORES)))
    return np.stack([res.results[b]["out"] for b in range(B)], axis=0)
